# revision 11
# baseline (speedup 1.0000x reference)
"""NeuralCDE (RK4 over cubic-spline control path) Trainium2 kernel.

Strategy: data-parallel over batch (2048 -> 8 cores x 256). Per core, the
256-batch is split into two independent 128-wide "streams" software-
pipelined against each other so the serial RK4 chain of one stream hides
behind engine work of the other.

All activations live transposed (feature on partitions, batch on free dim):
  zT (64, 128/stream) -> mm1: hT = relu(W1 @ zT)        [PE, K=64]
  mm2: gT = tanh(W2 @ hT)  as 8 chunks of (128, 128)    [PE, K=128]
  einsum bhc,bc->bh: prod = gT * rep(dX) [DVE broadcast mul], then two
  sets of 8 accumulating "selector" matmuls on PE reduce partition groups
  of 16: one alpha-scaled (k for the next RK4 stage input) and one
  beta-scaled accumulating zdelta = sum_i beta_i k_i across the whole step
  (the RK4 combine, done entirely on PE). Selectors are zero-padded to
  M=128 so FWL hides their weight loads.

Emission is organized in "ticks" (one stream-eval each). Each tick emits:
  PE:  mm1_X, mm2_X x8, deferred zdelta-sels_X, [consumer: ksel_Y x8]
  DVE: relu_X, [consumer: rhs_Y = k+z], mul_X
  ACT: tanh_X
so each engine's in-order queue matches data-readiness order and the two
streams stay half-period out of phase.

Spline derivative tables (d0/dm/d1 at integer/half knots are linear combos
of the Hermite coeffs) are precomputed on host, transposed to (16, B),
partition-replicated to (128, B), cast bf16, and streamed per step by DMA.

Matmul inputs are bf16 (fp32 PSUM accumulation); the z state stays fp32.
LDWEIGHTS fillers keep the PE HAM activity monitor busy so the PE clock
stays at 2.4 GHz instead of the idle-throttled 1.2 GHz.

PSUM budget (8 banks): per stream: gps (2 banks) + combo (sel out @ cols
0:128 + mm1 out @ cols 256:384 packed in 1 bank; single-matmul groups may
share a bank with a finished accumulation group) + zdelta (1 bank).
"""

import numpy as np
import ml_dtypes

import concourse.bass as bass
import concourse.tile as tile
from concourse import bacc, mybir
from concourse import bass_utils

BF16 = ml_dtypes.bfloat16

# problem dims (hardcoded per contest contract)
C = 16
H = 64
HID = 128
OUT = 1
L = 128
B = 2048
NCORES = 8
BL = B // NCORES          # 256 batch per core
BS = BL // 2              # 128 per stream
NCH = 8                   # chunks of 128 rows of (H*C)
NSTEP = L - 1             # 127
NSLOT = 2 * NSTEP + 1     # 127 b-slots + 1 final-deriv slot + 127 dm-slots

SEL_K = {0: 0, 1: 0, 2: 1}        # e -> selector scale block (0.5S, S)
SEL_Z = {0: 2, 1: 3, 2: 3, 3: 2}  # e -> (S/6, S/3)

LDW_FILLERS = 4                   # PE warmth fillers per tick

_cache = {}


def _build_nc(use_b2: bool):
    fp32 = mybir.dt.float32
    bf16 = mybir.dt.bfloat16
    AF = mybir.ActivationFunctionType
    OP = mybir.AluOpType

    nc = bacc.Bacc("TRN2", target_bir_lowering=False, debug=False)

    rep_d = nc.dram_tensor("rep", [NSLOT, 128, BL], bf16, kind="ExternalInput").ap()
    x0t_d = nc.dram_tensor("x0t", [C, BL], bf16, kind="ExternalInput").ap()
    w1t_d = nc.dram_tensor("w1t", [H, HID], bf16, kind="ExternalInput").ap()
    w2t_d = nc.dram_tensor("w2t", [HID, H * C], bf16, kind="ExternalInput").ap()
    wit_d = nc.dram_tensor("wit", [C, H], bf16, kind="ExternalInput").ap()
    wot_d = nc.dram_tensor("wot", [H, OUT], bf16, kind="ExternalInput").ap()
    # 4 scale blocks x 8 per-chunk selectors, zero-padded to M=128 for FWL
    sel_d = nc.dram_tensor("sel", [128, 4 * NCH * 128], bf16, kind="ExternalInput").ap()
    b1_d = nc.dram_tensor("b1c", [HID, 1], fp32, kind="ExternalInput").ap()
    bi_d = nc.dram_tensor("bic", [H, 1], fp32, kind="ExternalInput").ap()
    bo_d = nc.dram_tensor("boc", [OUT, 1], fp32, kind="ExternalInput").ap()
    b2_d = nc.dram_tensor("b2c", [128, NCH], fp32, kind="ExternalInput").ap()
    out_d = nc.dram_tensor("out", [OUT, BL], fp32, kind="ExternalOutput").ap()

    with tile.TileContext(nc) as tc:
        with (
            tc.tile_pool(name="consts", bufs=1) as consts,
            tc.tile_pool(name="bslot", bufs=4) as bslot,
            tc.tile_pool(name="dmslot", bufs=3) as dmslot,
            tc.tile_pool(name="state", bufs=2) as state,
            tc.tile_pool(name="work", bufs=2) as work,
            tc.tile_pool(name="psA", bufs=1, space="PSUM") as psA,
            tc.tile_pool(name="psB", bufs=1, space="PSUM") as psB,
        ):
            ps = {0: psA, 1: psB}

            w1t = consts.tile([H, HID], bf16, tag="w1t")
            nc.sync.dma_start(w1t[:], w1t_d)
            w2t = consts.tile([HID, H * C], bf16, tag="w2t")
            nc.sync.dma_start(w2t[:], w2t_d)
            wit = consts.tile([C, H], bf16, tag="wit")
            nc.sync.dma_start(wit[:], wit_d)
            wot = consts.tile([H, OUT], bf16, tag="wot")
            nc.sync.dma_start(wot[:], wot_d)
            selt = consts.tile([128, 4 * NCH * 128], bf16, tag="sel")
            nc.sync.dma_start(selt[:], sel_d)
            b1t = consts.tile([HID, 1], fp32, tag="b1")
            nc.sync.dma_start(b1t[:], b1_d)
            bit = consts.tile([H, 1], fp32, tag="bi")
            nc.sync.dma_start(bit[:], bi_d)
            bot = consts.tile([OUT, 1], fp32, tag="bo")
            nc.sync.dma_start(bot[:], bo_d)
            if use_b2:
                b2t = consts.tile([128, NCH], fp32, tag="b2")
                nc.sync.dma_start(b2t[:], b2_d)
            x0t = consts.tile([C, BL], bf16, tag="x0t")
            nc.sync.dma_start(x0t[:], x0t_d)

            def filler(n=1):
                for _ in range(n):
                    nc.tensor.ldweights(w2t[:, 0:128])

            def selcol(block, j):
                return (block * NCH + j) * 128

            # init: z0 = W_init @ X0T + b_init  (64, 256) -> split per stream
            z0ps = ps[0].tile([128, 384], fp32, tag="combo0", name="z0ps")
            nc.tensor.matmul(z0ps[0:H, 0:BL], wit[:], x0t[:], start=True, stop=True)
            z = {}
            rhs = {}
            for s in range(2):
                cs = slice(s * BS, (s + 1) * BS)
                z[s] = state.tile([H, BS], fp32, tag=f"z{s}", name=f"z{s}")
                nc.scalar.activation(z[s][:], z0ps[0:H, cs], AF.Identity, bias=bit[:])
                rhs[s] = work.tile([H, BS], bf16, tag=f"rhs{s}", name=f"rhs{s}")
                nc.vector.tensor_copy(rhs[s][:], z[s][:])

            # spline slot prefetch
            btiles = {}
            dmtiles = {}

            def load_b(i):
                t = bslot.tile([128, BL], bf16, tag="b", name="bt")
                nc.sync.dma_start(t[:], rep_d[i])
                btiles[i] = t

            def load_dm(i):
                t = dmslot.tile([128, BL], bf16, tag="dm", name="dmt")
                nc.sync.dma_start(t[:], rep_d[NSTEP + 1 + i])
                dmtiles[i] = t

            load_b(0)
            load_b(1)
            load_dm(0)
            load_b(2)
            load_dm(1)

            combo = {}
            prod = {}
            zdelta = {}
            zsel_pending = {0: [], 1: []}

            def producer_a(s, e, D):
                """mm1 + relu + mm2 + deferred zdelta-sels of stream s."""
                cs = slice(s * BS, (s + 1) * BS)
                if e == 0:
                    zdelta[s] = ps[s].tile(
                        [128, BS], fp32, tag=f"zd{s}", name=f"zd{s}"
                    )
                combo[s] = ps[s].tile(
                    [128, 384], fp32, tag=f"combo{s}", name=f"combo{s}"
                )
                mm1 = combo[s][:, 256:384]
                nc.tensor.matmul(mm1, w1t[:], rhs[s][:], start=True, stop=True)
                h = work.tile([HID, BS], bf16, tag=f"h{s}", name=f"h{s}")
                nc.vector.tensor_scalar(h[:], mm1, b1t[:], 0.0, OP.add, OP.max)
                gps = ps[s].tile([128, NCH * BS], fp32, tag=f"g{s}", name=f"g{s}")
                for j in range(NCH):
                    nc.tensor.matmul(
                        gps[:, j * BS:(j + 1) * BS],
                        w2t[:, j * 128:(j + 1) * 128],
                        h[:],
                        start=True,
                        stop=True,
                    )
                for fn in zsel_pending[s]:
                    fn()
                zsel_pending[s].clear()
                filler(LDW_FILLERS)
                if use_b2:
                    for j in range(NCH):
                        nc.vector.tensor_scalar_add(
                            gps[:, j * BS:(j + 1) * BS],
                            gps[:, j * BS:(j + 1) * BS],
                            b2t[:, j:j + 1],
                        )
                return cs, gps, h

            def producer_b(s, e, D, cs, gps):
                """tanh + broadcast-mul of stream s."""
                g = work.tile([128, NCH * BS], bf16, tag=f"gs{s}", name=f"gs{s}")
                nc.scalar.activation(g[:], gps[:], AF.Tanh)
                p = work.tile([128, NCH * BS], bf16, tag=f"p{s}", name=f"p{s}")
                nc.vector.tensor_tensor(
                    p[:].rearrange("p (r f) -> p r f", r=NCH),
                    g[:].rearrange("p (r f) -> p r f", r=NCH),
                    D[:, cs].unsqueeze(1).broadcast_to([128, NCH, BS]),
                    OP.mult,
                )
                prod[s] = p

            def consumer(s, e):
                """k-selectors + rhs update (e<3) or zdelta finish + z' (e=3)."""
                if e < 3:
                    kb = SEL_K[e]
                    kp128 = combo[s][:, 0:BS]
                    for j in range(NCH):
                        c0 = selcol(kb, j)
                        nc.tensor.matmul(
                            kp128,
                            selt[:, c0:c0 + 128],
                            prod[s][:, j * BS:(j + 1) * BS],
                            start=(j == 0),
                            stop=(j == NCH - 1),
                        )
                    rnew = work.tile([H, BS], bf16, tag=f"rhs{s}", name=f"rhs{s}")
                    nc.vector.tensor_add(rnew[:], combo[s][0:H, 0:BS], z[s][:])
                    rhs[s] = rnew
                    zb = SEL_Z[e]
                    pr, zd, first = prod[s], zdelta[s], (e == 0)

                    def zsel(pr=pr, zd=zd, first=first, zb=zb):
                        for j in range(NCH):
                            c0 = selcol(zb, j)
                            nc.tensor.matmul(
                                zd[:],
                                selt[:, c0:c0 + 128],
                                pr[:, j * BS:(j + 1) * BS],
                                start=(first and j == 0),
                                stop=False,
                            )

                    zsel_pending[s].append(zsel)
                else:
                    # zdelta group finishes on the chain, then z' and rhs
                    zb = SEL_Z[3]
                    for j in range(NCH):
                        c0 = selcol(zb, j)
                        nc.tensor.matmul(
                            zdelta[s][:],
                            selt[:, c0:c0 + 128],
                            prod[s][:, j * BS:(j + 1) * BS],
                            start=False,
                            stop=(j == NCH - 1),
                        )
                    znew = state.tile([H, BS], fp32, tag=f"z{s}", name=f"zn{s}")
                    nc.vector.tensor_add(znew[:], zdelta[s][0:H, :], z[s][:])
                    z[s] = znew
                    rnew = work.tile([H, BS], bf16, tag=f"rhs{s}", name=f"rhs{s}")
                    nc.vector.tensor_copy(rnew[:], znew[:])
                    rhs[s] = rnew

            prev = None
            for n in range(NSTEP):
                if n + 3 <= NSTEP:
                    load_b(n + 3)
                if n + 2 <= NSTEP - 1:
                    load_dm(n + 2)
                dvec = (btiles[n], dmtiles[n], dmtiles[n], btiles[n + 1])
                for e in range(4):
                    D = dvec[e]
                    for s in range(2):
                        cs, gps, _h = producer_a(s, e, D)
                        if prev is not None:
                            consumer(*prev)
                        producer_b(s, e, D, cs, gps)
                        prev = (s, e)
            consumer(*prev)

            # output: outT = W_out @ zT + b_out
            ops_ = ps[0].tile([128, 384], fp32, tag="combo0", name="ops_")
            for s in range(2):
                cs = slice(s * BS, (s + 1) * BS)
                nc.tensor.matmul(ops_[0:OUT, cs], wot[:], rhs[s][:],
                                 start=True, stop=True)
            outs = work.tile([OUT, BL], fp32, tag="outs")
            nc.scalar.activation(outs[:], ops_[0:OUT, 0:BL], AF.Identity, bias=bot[:])
            nc.sync.dma_start(out_d, outs[:])

    nc.compile()
    return nc


def _host_prep(coeffs, W_init, b_init, W1, b1, W2, b2, W_out, b_out):
    """Build per-core input maps (host-side layout/precompute)."""
    coeffs = np.ascontiguousarray(coeffs, np.float32)
    b = coeffs[:, :, C:2 * C]
    tc_ = coeffs[:, :, 2 * C:3 * C]
    td = coeffs[:, :, 3 * C:4 * C]
    dm = b + np.float32(0.5) * tc_ + np.float32(0.25) * td         # (B, 127, C)
    dlast = b[:, L - 2] + tc_[:, L - 2] + td[:, L - 2]             # (B, C)

    # per-chunk selector: S_j[r, 8j + r//16] = scale, zero-padded to 128
    # columns; layout: col0(block, j) = (block*8 + j)*128
    r = np.arange(128)
    sel = np.zeros((128, 4 * NCH * 128), np.float32)
    for bi, scale in enumerate((0.5, 1.0, 1 / 6.0, 1 / 3.0)):
        for j in range(NCH):
            c0 = (bi * NCH + j) * 128
            sel[r, c0 + 8 * j + r // 16] = scale

    shared = {
        "w1t": np.ascontiguousarray(W1.T).astype(BF16),
        "w2t": np.ascontiguousarray(W2.T).astype(BF16),
        "wit": np.ascontiguousarray(W_init.T).astype(BF16),
        "wot": np.ascontiguousarray(W_out.T).astype(BF16),
        "sel": sel.astype(BF16),
        "b1c": np.ascontiguousarray(b1.reshape(HID, 1), np.float32),
        "bic": np.ascontiguousarray(b_init.reshape(H, 1), np.float32),
        "boc": np.ascontiguousarray(b_out.reshape(OUT, 1), np.float32),
        "b2c": np.ascontiguousarray(b2.reshape(NCH, 128).T, np.float32),
    }

    in_maps = []
    for c in range(NCORES):
        rows = slice(c * BL, (c + 1) * BL)
        rep = np.empty((NSLOT, C, BL), np.float32)
        rep[0:NSTEP] = b[rows].transpose(1, 2, 0)                  # slot n = b_n^T
        rep[NSTEP] = dlast[rows].T                                 # slot 127 = final d1
        rep[NSTEP + 1:] = dm[rows].transpose(1, 2, 0)              # slot 128+n = dm_n^T
        rep128 = np.tile(rep, (1, 128 // C, 1)).astype(BF16)       # (255,128,256)
        x0t = np.ascontiguousarray(coeffs[rows, 0, 0:C].T).astype(BF16)
        m = dict(shared)
        m["rep"] = rep128
        m["x0t"] = x0t
        in_maps.append(m)
    return in_maps


def kernel(coeffs, W_init, b_init, W1, b1, W2, b2, W_out, b_out):
    use_b2 = bool(np.any(np.asarray(b2)))
    key = ("nc", use_b2)
    if key not in _cache:
        _cache[key] = _build_nc(use_b2)
    nc = _cache[key]

    in_maps = _host_prep(coeffs, W_init, b_init, W1, b1, W2, b2, W_out, b_out)
    res = bass_utils.run_bass_kernel_spmd(nc, in_maps, core_ids=list(range(NCORES)))
    out = np.concatenate(
        [res.results[c]["out"].reshape(BL, OUT) for c in range(NCORES)], axis=0
    )
    return out.astype(np.float32)


if __name__ == "__main__":
    data = np.load("/root/problem/golden.npz")
    ins = {k: data[k] for k in data.files if k != "expected"}
    got = kernel(**ins)
    exp = data["expected"]
    rel = np.abs(got - exp).max() / np.abs(exp).max()
    print("Relative error:", rel)


# revision 13
# speedup vs baseline: 1.0683x; 1.0683x over previous
"""NeuralCDE (RK4 over cubic-spline control path) Trainium2 kernel.

Strategy: data-parallel over batch (2048 -> 8 cores x 256). Per core, the
256-batch is split into two independent 128-wide "streams" software-
pipelined against each other so the serial RK4 chain of one stream hides
behind engine work of the other.

All activations live transposed (feature on partitions, batch on free dim):
  zT (64, 128/stream) -> mm1: hT = relu(W1 @ zT)        [PE, K=64]
  mm2: gT = tanh(W2 @ hT)  as 8 chunks of (128, 128)    [PE, K=128]
  einsum bhc,bc->bh: prod = gT * rep(dX) [DVE broadcast mul], then two
  sets of 8 accumulating "selector" matmuls on PE reduce partition groups
  of 16: one alpha-scaled (k for the next RK4 stage input) and one
  beta-scaled accumulating zdelta = sum_i beta_i k_i across the whole step
  (the RK4 combine, done entirely on PE). Selectors are zero-padded to
  M=128 so FWL hides their weight loads.

Emission is organized in "ticks" (one stream-eval each). Each tick emits:
  PE:  mm1_X, mm2_X x8, deferred zdelta-sels_X, [consumer: ksel_Y x8]
  DVE: relu_X, [consumer: rhs_Y = k+z], mul_X
  ACT: tanh_X
so each engine's in-order queue matches data-readiness order and the two
streams stay half-period out of phase.

Spline derivative tables (d0/dm/d1 at integer/half knots are linear combos
of the Hermite coeffs) are precomputed on host, transposed to (16, B),
partition-replicated to (128, B), cast bf16, and streamed per step by DMA.

Matmul inputs are bf16 (fp32 PSUM accumulation); the z state stays fp32.
LDWEIGHTS fillers keep the PE HAM activity monitor busy so the PE clock
stays at 2.4 GHz instead of the idle-throttled 1.2 GHz.

PSUM budget (8 banks): per stream: gps (2 banks) + combo (sel out @ cols
0:128 + mm1 out @ cols 256:384 packed in 1 bank; single-matmul groups may
share a bank with a finished accumulation group) + zdelta (1 bank).
"""

import numpy as np
import ml_dtypes

import concourse.bass as bass
import concourse.tile as tile
from concourse import bacc, mybir
from concourse import bass_utils

BF16 = ml_dtypes.bfloat16

# problem dims (hardcoded per contest contract)
C = 16
H = 64
HID = 128
OUT = 1
L = 128
B = 2048
NCORES = 8
BL = B // NCORES          # 256 batch per core
BS = BL // 2              # 128 per stream
NCH = 8                   # chunks of 128 rows of (H*C)
NSTEP = L - 1             # 127
NSLOT = 2 * NSTEP + 1     # 127 b-slots + 1 final-deriv slot + 127 dm-slots

SEL_K = {0: 0, 1: 0, 2: 1}        # e -> selector scale block (0.5S, S)
SEL_Z = {0: 2, 1: 3, 2: 3, 3: 2}  # e -> (S/6, S/3)

LDW_FILLERS = 4                   # PE warmth fillers per tick

_cache = {}


def _build_nc(use_b2: bool):
    fp32 = mybir.dt.float32
    bf16 = mybir.dt.bfloat16
    AF = mybir.ActivationFunctionType
    OP = mybir.AluOpType

    nc = bacc.Bacc("TRN2", target_bir_lowering=False, debug=False)

    rep_d = nc.dram_tensor("rep", [NSLOT, 128, BL], bf16, kind="ExternalInput").ap()
    x0t_d = nc.dram_tensor("x0t", [C, BL], bf16, kind="ExternalInput").ap()
    w1t_d = nc.dram_tensor("w1t", [H, HID], bf16, kind="ExternalInput").ap()
    w2t_d = nc.dram_tensor("w2t", [HID, H * C], bf16, kind="ExternalInput").ap()
    wit_d = nc.dram_tensor("wit", [C, H], bf16, kind="ExternalInput").ap()
    wot_d = nc.dram_tensor("wot", [H, OUT], bf16, kind="ExternalInput").ap()
    # 4 scale blocks x 8 per-chunk selectors, zero-padded to M=128 for FWL
    sel_d = nc.dram_tensor("sel", [128, 4 * NCH * 128], bf16, kind="ExternalInput").ap()
    b1_d = nc.dram_tensor("b1c", [HID, 1], fp32, kind="ExternalInput").ap()
    bi_d = nc.dram_tensor("bic", [H, 1], fp32, kind="ExternalInput").ap()
    bo_d = nc.dram_tensor("boc", [OUT, 1], fp32, kind="ExternalInput").ap()
    b2_d = nc.dram_tensor("b2c", [128, NCH], fp32, kind="ExternalInput").ap()
    out_d = nc.dram_tensor("out", [OUT, BL], fp32, kind="ExternalOutput").ap()

    with tile.TileContext(nc) as tc:
        with (
            tc.tile_pool(name="consts", bufs=1) as consts,
            tc.tile_pool(name="bslot", bufs=4) as bslot,
            tc.tile_pool(name="dmslot", bufs=3) as dmslot,
            tc.tile_pool(name="state", bufs=2) as state,
            tc.tile_pool(name="work", bufs=2) as work,
            tc.tile_pool(name="psA", bufs=1, space="PSUM") as psA,
            tc.tile_pool(name="psB", bufs=1, space="PSUM") as psB,
        ):
            ps = {0: psA, 1: psB}

            w1t = consts.tile([H, HID], bf16, tag="w1t")
            nc.sync.dma_start(w1t[:], w1t_d)
            w2t = consts.tile([HID, H * C], bf16, tag="w2t")
            nc.sync.dma_start(w2t[:], w2t_d)
            wit = consts.tile([C, H], bf16, tag="wit")
            nc.sync.dma_start(wit[:], wit_d)
            wot = consts.tile([H, OUT], bf16, tag="wot")
            nc.sync.dma_start(wot[:], wot_d)
            selt = consts.tile([128, 4 * NCH * 128], bf16, tag="sel")
            nc.sync.dma_start(selt[:], sel_d)
            b1t = consts.tile([HID, 1], fp32, tag="b1")
            nc.sync.dma_start(b1t[:], b1_d)
            bit = consts.tile([H, 1], fp32, tag="bi")
            nc.sync.dma_start(bit[:], bi_d)
            bot = consts.tile([OUT, 1], fp32, tag="bo")
            nc.sync.dma_start(bot[:], bo_d)
            if use_b2:
                b2t = consts.tile([128, NCH], fp32, tag="b2")
                nc.sync.dma_start(b2t[:], b2_d)
            x0t = consts.tile([C, BL], bf16, tag="x0t")
            nc.sync.dma_start(x0t[:], x0t_d)

            def selcol(block, j):
                return (block * NCH + j) * 128

            # init: z0 = W_init @ X0T + b_init  (64, 256) -> split per stream
            z0ps = ps[0].tile([128, 384], fp32, tag="combo0", name="z0ps")
            nc.tensor.matmul(z0ps[0:H, 0:BL], wit[:], x0t[:], start=True, stop=True)
            z = {}
            rhs = {}
            for s in range(2):
                cs = slice(s * BS, (s + 1) * BS)
                z[s] = state.tile([H, BS], fp32, tag=f"z{s}", name=f"z{s}")
                nc.scalar.activation(z[s][:], z0ps[0:H, cs], AF.Identity, bias=bit[:])
                rhs[s] = work.tile([H, BS], bf16, tag=f"rhs{s}", name=f"rhs{s}")
                nc.vector.tensor_copy(rhs[s][:], z[s][:])

            # spline slot prefetch
            btiles = {}
            dmtiles = {}

            def load_b(i):
                t = bslot.tile([128, BL], bf16, tag="b", name="bt")
                nc.sync.dma_start(t[:], rep_d[i])
                btiles[i] = t

            def load_dm(i):
                t = dmslot.tile([128, BL], bf16, tag="dm", name="dmt")
                nc.sync.dma_start(t[:], rep_d[NSTEP + 1 + i])
                dmtiles[i] = t

            load_b(0)
            load_b(1)
            load_dm(0)
            load_b(2)
            load_dm(1)

            combo = {}
            prod = {}
            zdelta = {}
            zsel_pending = {0: [], 1: []}

            def producer_a(s, e, D):
                """mm1 + relu + mm2 + deferred zdelta-sels of stream s."""
                cs = slice(s * BS, (s + 1) * BS)
                if e == 0:
                    zdelta[s] = ps[s].tile(
                        [128, BS], fp32, tag=f"zd{s}", name=f"zd{s}"
                    )
                combo[s] = ps[s].tile(
                    [128, 384], fp32, tag=f"combo{s}", name=f"combo{s}"
                )
                mm1 = combo[s][:, 256:384]
                nc.tensor.matmul(mm1, w1t[:], rhs[s][:], start=True, stop=True)
                h = work.tile([HID, BS], bf16, tag=f"h{s}", name=f"h{s}")
                nc.vector.tensor_scalar(h[:], mm1, b1t[:], 0.0, OP.add, OP.max)
                # deferred zdelta-sels fill the PE bubble while relu runs
                for fn in zsel_pending[s]:
                    fn()
                zsel_pending[s].clear()
                gps = ps[s].tile([128, NCH * BS], fp32, tag=f"g{s}", name=f"g{s}")
                for j in range(NCH):
                    nc.tensor.matmul(
                        gps[:, j * BS:(j + 1) * BS],
                        w2t[:, j * 128:(j + 1) * 128],
                        h[:],
                        start=True,
                        stop=True,
                    )
                if use_b2:
                    for j in range(NCH):
                        nc.vector.tensor_scalar_add(
                            gps[:, j * BS:(j + 1) * BS],
                            gps[:, j * BS:(j + 1) * BS],
                            b2t[:, j:j + 1],
                        )
                return cs, gps, h

            def producer_b(s, e, D, cs, gps):
                """tanh + broadcast-mul of stream s."""
                g = work.tile([128, NCH * BS], bf16, tag=f"gs{s}", name=f"gs{s}")
                nc.scalar.activation(g[:], gps[:], AF.Tanh)
                p = work.tile([128, NCH * BS], bf16, tag=f"p{s}", name=f"p{s}")
                nc.vector.tensor_tensor(
                    p[:].rearrange("p (r f) -> p r f", r=NCH),
                    g[:].rearrange("p (r f) -> p r f", r=NCH),
                    D[:, cs].unsqueeze(1).broadcast_to([128, NCH, BS]),
                    OP.mult,
                )
                prod[s] = p

            def consumer(s, e):
                """k-selectors + rhs update (e<3) or zdelta finish + z' (e=3)."""
                if e < 3:
                    kb = SEL_K[e]
                    kp128 = combo[s][:, 0:BS]
                    for j in range(NCH):
                        c0 = selcol(kb, j)
                        nc.tensor.matmul(
                            kp128,
                            selt[:, c0:c0 + 128],
                            prod[s][:, j * BS:(j + 1) * BS],
                            start=(j == 0),
                            stop=(j == NCH - 1),
                        )
                    rnew = work.tile([H, BS], bf16, tag=f"rhs{s}", name=f"rhs{s}")
                    nc.vector.tensor_add(rnew[:], combo[s][0:H, 0:BS], z[s][:])
                    rhs[s] = rnew
                    zb = SEL_Z[e]
                    pr, zd, first = prod[s], zdelta[s], (e == 0)

                    def zsel(pr=pr, zd=zd, first=first, zb=zb):
                        for j in range(NCH):
                            c0 = selcol(zb, j)
                            nc.tensor.matmul(
                                zd[:],
                                selt[:, c0:c0 + 128],
                                pr[:, j * BS:(j + 1) * BS],
                                start=(first and j == 0),
                                stop=False,
                            )

                    zsel_pending[s].append(zsel)
                else:
                    # zdelta group finishes on the chain, then z' and rhs
                    zb = SEL_Z[3]
                    for j in range(NCH):
                        c0 = selcol(zb, j)
                        nc.tensor.matmul(
                            zdelta[s][:],
                            selt[:, c0:c0 + 128],
                            prod[s][:, j * BS:(j + 1) * BS],
                            start=False,
                            stop=(j == NCH - 1),
                        )
                    znew = state.tile([H, BS], fp32, tag=f"z{s}", name=f"zn{s}")
                    nc.vector.tensor_add(znew[:], zdelta[s][0:H, :], z[s][:])
                    z[s] = znew
                    rnew = work.tile([H, BS], bf16, tag=f"rhs{s}", name=f"rhs{s}")
                    nc.vector.tensor_copy(rnew[:], znew[:])
                    rhs[s] = rnew

            prev = None
            for n in range(NSTEP):
                if n + 3 <= NSTEP:
                    load_b(n + 3)
                if n + 2 <= NSTEP - 1:
                    load_dm(n + 2)
                dvec = (btiles[n], dmtiles[n], dmtiles[n], btiles[n + 1])
                for e in range(4):
                    D = dvec[e]
                    for s in range(2):
                        cs, gps, _h = producer_a(s, e, D)
                        if prev is not None:
                            consumer(*prev)
                        producer_b(s, e, D, cs, gps)
                        prev = (s, e)
            consumer(*prev)

            # output: outT = W_out @ zT + b_out
            ops_ = ps[0].tile([128, 384], fp32, tag="combo0", name="ops_")
            for s in range(2):
                cs = slice(s * BS, (s + 1) * BS)
                nc.tensor.matmul(ops_[0:OUT, cs], wot[:], rhs[s][:],
                                 start=True, stop=True)
            outs = work.tile([OUT, BL], fp32, tag="outs")
            nc.scalar.activation(outs[:], ops_[0:OUT, 0:BL], AF.Identity, bias=bot[:])
            nc.sync.dma_start(out_d, outs[:])

    nc.compile()
    return nc


def _host_prep(coeffs, W_init, b_init, W1, b1, W2, b2, W_out, b_out):
    """Build per-core input maps (host-side layout/precompute)."""
    coeffs = np.ascontiguousarray(coeffs, np.float32)
    b = coeffs[:, :, C:2 * C]
    tc_ = coeffs[:, :, 2 * C:3 * C]
    td = coeffs[:, :, 3 * C:4 * C]
    dm = b + np.float32(0.5) * tc_ + np.float32(0.25) * td         # (B, 127, C)
    dlast = b[:, L - 2] + tc_[:, L - 2] + td[:, L - 2]             # (B, C)

    # per-chunk selector: S_j[r, 8j + r//16] = scale, zero-padded to 128
    # columns; layout: col0(block, j) = (block*8 + j)*128
    r = np.arange(128)
    sel = np.zeros((128, 4 * NCH * 128), np.float32)
    for bi, scale in enumerate((0.5, 1.0, 1 / 6.0, 1 / 3.0)):
        for j in range(NCH):
            c0 = (bi * NCH + j) * 128
            sel[r, c0 + 8 * j + r // 16] = scale

    shared = {
        "w1t": np.ascontiguousarray(W1.T).astype(BF16),
        "w2t": np.ascontiguousarray(W2.T).astype(BF16),
        "wit": np.ascontiguousarray(W_init.T).astype(BF16),
        "wot": np.ascontiguousarray(W_out.T).astype(BF16),
        "sel": sel.astype(BF16),
        "b1c": np.ascontiguousarray(b1.reshape(HID, 1), np.float32),
        "bic": np.ascontiguousarray(b_init.reshape(H, 1), np.float32),
        "boc": np.ascontiguousarray(b_out.reshape(OUT, 1), np.float32),
        "b2c": np.ascontiguousarray(b2.reshape(NCH, 128).T, np.float32),
    }

    in_maps = []
    for c in range(NCORES):
        rows = slice(c * BL, (c + 1) * BL)
        rep = np.empty((NSLOT, C, BL), np.float32)
        rep[0:NSTEP] = b[rows].transpose(1, 2, 0)                  # slot n = b_n^T
        rep[NSTEP] = dlast[rows].T                                 # slot 127 = final d1
        rep[NSTEP + 1:] = dm[rows].transpose(1, 2, 0)              # slot 128+n = dm_n^T
        rep128 = np.tile(rep, (1, 128 // C, 1)).astype(BF16)       # (255,128,256)
        x0t = np.ascontiguousarray(coeffs[rows, 0, 0:C].T).astype(BF16)
        m = dict(shared)
        m["rep"] = rep128
        m["x0t"] = x0t
        in_maps.append(m)
    return in_maps


def kernel(coeffs, W_init, b_init, W1, b1, W2, b2, W_out, b_out):
    use_b2 = bool(np.any(np.asarray(b2)))
    key = ("nc", use_b2)
    if key not in _cache:
        _cache[key] = _build_nc(use_b2)
    nc = _cache[key]

    in_maps = _host_prep(coeffs, W_init, b_init, W1, b1, W2, b2, W_out, b_out)
    res = bass_utils.run_bass_kernel_spmd(nc, in_maps, core_ids=list(range(NCORES)))
    out = np.concatenate(
        [res.results[c]["out"].reshape(BL, OUT) for c in range(NCORES)], axis=0
    )
    return out.astype(np.float32)


if __name__ == "__main__":
    data = np.load("/root/problem/golden.npz")
    ins = {k: data[k] for k in data.files if k != "expected"}
    got = kernel(**ins)
    exp = data["expected"]
    rel = np.abs(got - exp).max() / np.abs(exp).max()
    print("Relative error:", rel)
